# revision 21
# baseline (speedup 1.0000x reference)
"""Trainium2 Bass kernel for nn_KGAT_80590766342918 (KGAT attention message passing).

Reference computation (B=1024, N=50, K=5, D=ATT=128):
    concat  = [ent.broadcast_k, ne, nr]            # [B,N,K,3D]
    h       = concat @ W1 + b1                      # [B,N,K,ATT]
    logits  = h @ W2 + b2                           # [B,N,K,1]
    att     = softmax_k(logits)
    out     = [ent, sum_k att*ne]                   # [B,N,2D]

There is no nonlinearity between fc1 and fc2, so the MLP collapses to a
single 384-dim dot product per (b,n,k):
    logits = concat @ (W1 @ W2) + (b1 @ W2 + b2)
and softmax over k is invariant to per-(b,n) constant shifts, so the
ent-dependent term and all biases drop out entirely:
    att = softmax_k(ne_k . w_ne  +  nr_k . w_nr)
with w_ne = (W1@W2)[D:2D, 0], w_nr = (W1@W2)[2D:3D, 0].

Sharding: pure data parallel over B across 8 cores (B=128 per core, i.e.
6400 (b,n)-rows per core). Rows are placed on SBUF partitions; the dot
products reduce along the free dim via the fused DVE tensor_tensor_reduce.
"""

import os
import sys

import numpy as np

for _p in ("/opt/trn_rl_repo",):
    if _p not in sys.path and os.path.isdir(_p):
        sys.path.append(_p)

import concourse.bass as bass
import concourse.tile as tile
from concourse import mybir
from concourse.bass_utils import run_bass_kernel_spmd

B, N, K, D = 1024, 50, 5, 128
NCORES = 8
P = 128                      # SBUF partitions = rows per tile
ROWS = (B // NCORES) * N     # 6400 rows per core
KD = K * D                   # 640
F32 = mybir.dt.float32


def build_nc(rows: int = ROWS) -> bass.Bass:
    ntiles = rows // P
    nc = bass.Bass()
    ent = nc.dram_tensor("ent", [rows, D], F32, kind="ExternalInput")
    # host-interleaved [rows, K, 2D]: per k, ne_k then nr_k — one DMA per
    # tile, and each fused dot reads one contiguous [P, 2D] slice against
    # [w_ne | w_nr]
    netr_in = nc.dram_tensor("netr", [rows, 2 * KD], F32, kind="ExternalInput")
    w12 = nc.dram_tensor("w12", [P, 2 * D], F32, kind="ExternalInput")
    # two output tensors (host concatenates): a single [rows, 2D] output
    # would WAW-couple every store to the entity passthrough DMA, pushing
    # stores over this walrus's one-sync-wait-per-instruction limit
    out_ent = nc.dram_tensor("out_ent", [rows, D], F32, kind="ExternalOutput")
    # one attention-output tensor PER TILE: distinct DRAM tensors carry no
    # WAW dep, so stores never chain waits across DMA lanes
    out_atts = [
        nc.dram_tensor(f"out_att{i}", [P, D], F32, kind="ExternalOutput")
        for i in range(ntiles)
    ]

    with tile.TileContext(nc) as tc:
        with (
            tc.tile_pool(name="const", bufs=1) as const_pool,
            tc.tile_pool(name="io", bufs=8) as io_pool,
            # bufs=ntiles: every per-tile temp gets a fresh slot, so no
            # WAR/WAW slot-reuse waits are ever emitted (wait-limit again)
            tc.tile_pool(name="work", bufs=ntiles) as work_pool,
        ):
            w12_t = const_pool.tile([P, 2 * D], F32)
            nc.sync.dma_start(out=w12_t[:], in_=w12[:, :])

            # entity passthrough: one big DRAM->DRAM copy
            nc.sync.dma_start(out=out_ent[:, :], in_=ent[:, :])

            for i in range(ntiles):
                r0 = i * P
                netr = io_pool.tile([P, 2 * KD], F32)
                nc.sync.dma_start(out=netr[:], in_=netr_in[r0 : r0 + P, :])

                # wait-soaker: absorb the DMA wait on a cheap copy so the STT
                # ops below each need at most one sync wait (this walrus
                # rejects instructions with several waits). DVE is the ONLY
                # engine reading netr, so the slot-reuse DMA also needs just
                # one wait.
                dve_tmp = work_pool.tile([P, 2], F32)
                nc.vector.tensor_copy(dve_tmp[:], netr[:, 0:2])

                # logits[:, k] = ne_k . w_ne + nr_k . w_nr  (fused mul+reduce;
                # the elementwise product output is discarded via a stride-0
                # broadcast AP)
                logits = work_pool.tile([P, K], F32)
                scratch = work_pool.tile([P, 1], F32)
                for k in range(K):
                    nc.vector.scalar_tensor_tensor(
                        out=scratch.broadcast_to((P, 2 * D)),
                        in0=netr[:, k * 2 * D : (k + 1) * 2 * D],
                        scalar=1.0,
                        in1=w12_t[:],
                        op0=mybir.AluOpType.mult,
                        op1=mybir.AluOpType.mult,
                        accum_out=logits[:, k : k + 1],
                    )

                # softmax over k (free dim, 5 wide)
                negmax = work_pool.tile([P, 1], F32)
                nc.vector.tensor_reduce(
                    out=negmax[:],
                    in_=logits[:],
                    axis=mybir.AxisListType.X,
                    op=mybir.AluOpType.max,
                    negate=True,
                )
                exps = work_pool.tile([P, K], F32)
                sumexp = work_pool.tile([P, 1], F32)
                nc.scalar.activation(
                    out=exps[:],
                    in_=logits[:],
                    func=mybir.ActivationFunctionType.Exp,
                    bias=negmax[:],
                    scale=1.0,
                    accum_out=sumexp[:],
                )
                recip = work_pool.tile([P, 1], F32)
                nc.vector.reciprocal(recip[:], sumexp[:])
                att = work_pool.tile([P, K], F32)
                nc.vector.tensor_scalar_mul(att[:], exps[:], recip[:])

                # out2 = sum_k att_k * ne_k via a fused multiply-accumulate
                # chain: acc = (ne_k * att_k) + acc, ping-ponging two tiles
                acc_a = work_pool.tile([P, D], F32)
                acc_b = work_pool.tile([P, D], F32)
                accs = [acc_a, acc_b]
                nc.vector.tensor_scalar_mul(acc_a[:], netr[:, 0:D], att[:, 0:1])
                for k in range(1, K):
                    src = accs[(k - 1) % 2]
                    dst = accs[k % 2]
                    nc.vector.scalar_tensor_tensor(
                        out=dst[:],
                        in0=netr[:, k * 2 * D : k * 2 * D + D],
                        scalar=att[:, k : k + 1],
                        in1=src[:],
                        op0=mybir.AluOpType.mult,
                        op1=mybir.AluOpType.add,
                    )
                out2 = accs[(K - 1) % 2]
                nc.sync.dma_start(out=out_atts[i][:, :], in_=out2[:])

    _drop_redundant_lane_waits(nc)
    return nc


def _drop_redundant_lane_waits(nc: bass.Bass) -> None:
    """This walrus accepts only one sync-wait per instruction. Tile emits a
    data wait plus a DMA-lane flow wait on each DMA. The lane wait orders a
    DMA against the previous DMA on its sem lane — redundant here: all DMAs
    on a ring are issued by one engine and drain FIFO, sem counters are
    monotonic, and every data dep (RAW/WAR) is carried by the kept wait."""
    for bb in nc.m.functions[0].blocks:
        for inst in bb.instructions:
            si = inst.sync_info
            if si is None or si.on_wait is None or len(si.on_wait) <= 1:
                continue
            keep = [w for w in si.on_wait if not (
                "DMAHW" in w.ant_name or "DMASW" in w.ant_name)]
            lane = [w for w in si.on_wait if (
                "DMAHW" in w.ant_name or "DMASW" in w.ant_name)]
            if len(keep) > 1:
                # tail drain: DVE is the latest-finishing engine here and its
                # wait transitively covers ACT (DVE consumes ACT outputs)
                dve = [w for w in keep if "DVE" in w.ant_name]
                keep = dve[-1:] if dve else keep[-1:]
            if not keep:
                # keep the newest lane wait if nothing else remains
                keep = [max(lane, key=lambda w: w.wait_value)]
            assert len(keep) == 1, (inst.name, [w.ant_name for w in si.on_wait])
            si.on_wait = keep


_NC_CACHE: dict[int, bass.Bass] = {}


def make_in_maps(entity_embedding, neigh_entity_embedding, neigh_relation_embedding, W1, W2):
    w = (np.asarray(W1, np.float32) @ np.asarray(W2, np.float32))[:, 0]  # [3D]
    w12_row = np.concatenate([w[D : 2 * D], w[2 * D : 3 * D]])           # [2D]
    w12 = np.ascontiguousarray(np.broadcast_to(w12_row, (P, 2 * D)), np.float32)

    ent = np.ascontiguousarray(entity_embedding, np.float32)
    ne = np.asarray(neigh_entity_embedding, np.float32)
    nr = np.asarray(neigh_relation_embedding, np.float32)
    # interleave per k: [B, N, K, 2, D] so each (b,n) row is [ne_0|nr_0|ne_1|...]
    netr = np.empty((B, N, K, 2, D), np.float32)
    netr[:, :, :, 0, :] = ne
    netr[:, :, :, 1, :] = nr

    bs = B // NCORES
    in_maps = []
    for c in range(NCORES):
        sl = slice(c * bs, (c + 1) * bs)
        in_maps.append(
            {
                "ent": ent[sl].reshape(ROWS, D),
                "netr": netr[sl].reshape(ROWS, 2 * KD),
                "w12": w12,
            }
        )
    return in_maps


def kernel(
    entity_embedding,
    neigh_entity_embedding,
    neigh_relation_embedding,
    W1,
    b1,
    W2,
    b2,
):
    # b1/b2 and the entity term only shift logits per-(b,n); softmax over k
    # is invariant to them, so they are unused.
    in_maps = make_in_maps(
        entity_embedding, neigh_entity_embedding, neigh_relation_embedding, W1, W2
    )
    if ROWS not in _NC_CACHE:
        _NC_CACHE[ROWS] = build_nc(ROWS)
    nc = _NC_CACHE[ROWS]
    res = run_bass_kernel_spmd(nc, in_maps, list(range(NCORES))).results
    bs = B // NCORES
    out = np.empty((B, N, 2 * D), np.float32)
    flat = out.reshape(B * N, 2 * D)
    for c, r in enumerate(res):
        out[c * bs : (c + 1) * bs, :, 0:D] = np.asarray(r["out_ent"]).reshape(
            bs, N, D
        )
        for i in range(ROWS // P):
            r0 = c * ROWS + i * P
            flat[r0 : r0 + P, D : 2 * D] = np.asarray(r[f"out_att{i}"])
    return out
